# revision 2
# baseline (speedup 1.0000x reference)
"""Bass/Trainium2 kernel for nn_LowRankLoss.

Reference computation:
  m      = mean(feat, axis=1)                      # [n, h, w], channel mean
  normed = m / ||m||_F (per sample)
  rank   = #(singular values of normed > 0)        # [n]
  loss   = sum(max(0, -(rank1 - rank2))) / n

The memory-bound part (target_regime=memory) is the channel-mean reduction
over two [128, 256, 32, 64] f32 tensors (512 MiB total). That runs on 8
NeuronCores, data-parallel over the batch dim (16 samples/core). The device
returns per-sample channel sums [n, 2048]; the tiny per-sample SVDs
(128 matrices of 32x64) and the scalar loss are finished on host.

Device design per core (per input tensor, viewed [NS=16, 128, FF=4096]):
  - One fully contiguous 2 MiB DMA per sample -> SBUF [128, 4096];
    partition p holds channels (2p, 2p+1) back to back (16 KiB per
    partition row = one 16 KiB descriptor per partition, biggest
    descriptors the swizzle allows -> fewest packet switches). SWDGE
    (gpsimd) issues the steady-state input DMAs; the first two samples
    ride the HWDGE rings (sync/scalar, shorter first-byte) while the Q7
    SWDGE path spins up. The one-hot stationary loads AFTER the first
    input DMAs so it stays off the critical path.
  - VectorE folds the two channel halves (t[:, :F] + t[:, F:]) and rounds
    to fp32r for the PE (fp32r moving streams 1 cycle/row vs 4 for fp32).
  - TensorE reduces the remaining 128 channels (partition dim) per sample:
    stationary S_m [128, 8] is all-ones in column m = s%8 and zero
    elsewhere, so sample s lands in PSUM row m while other rows accumulate
    +0. Eight samples share one PSUM tile [8, F] (one accumulation group
    per 512-col bank chunk).
  - acc [8, F] -> SBUF via VectorE -> 64 KiB DMA out per group.
fp32r truncates the data mantissa (~1e-4 rel err), far below what could
flip a singular-value-positivity count (min sigma ~2e-2 here).
"""

import numpy as np

N_CORES = 8
NS = 16           # samples per core
C = 256           # channels
H, W = 32, 64
F = H * W         # 2048 spatial
FF = 2 * F        # free cols per sample tile (two channels per partition)
P = 128           # partitions
SG = 8            # samples per PSUM group
NB = 4            # matmuls per sample (N=512 PSUM bank limit)
BN = F // NB      # 512

_CACHE = {}


def _build_nc():
    import concourse.bacc as bacc
    import concourse.mybir as mybir
    import concourse.tile as tile

    nc = bacc.Bacc(None, target_bir_lowering=False)
    f32 = mybir.dt.float32
    f32r = mybir.dt.float32r

    x_raw = nc.dram_tensor("x_raw", [NS, P, FF], f32, kind="ExternalInput")
    x_rect = nc.dram_tensor("x_rect", [NS, P, FF], f32, kind="ExternalInput")
    out_raw = nc.dram_tensor("out_raw", [NS, F], f32, kind="ExternalOutput")
    out_rect = nc.dram_tensor("out_rect", [NS, F], f32, kind="ExternalOutput")

    with tile.TileContext(nc) as tc:
        with (
            tc.tile_pool(name="io", bufs=7) as pool,
            tc.tile_pool(name="red", bufs=3) as redp,
            tc.tile_pool(name="small", bufs=2) as small,
            tc.tile_pool(name="psum", bufs=2, space="PSUM") as psum,
        ):
            # First two samples ride the HWDGE rings before anything else
            # so input bytes start flowing at the earliest possible point.
            t_first = []
            for s, eng in ((0, nc.sync), (1, nc.scalar)):
                t = pool.tile([P, FF], f32, tag="in")
                eng.dma_start(t[:], x_raw[s])
                t_first.append(t)

            # C[k, 8m + j] = 1 if j == m else 0; lhsT for sample s is the
            # [128, 8] slice C[:, 8m:8m+8] with m = s % 8.
            s_np = np.zeros((P, SG * SG), np.float32)
            for m in range(SG):
                s_np[:, SG * m + m] = 1.0
            s_dram = nc.inline_tensor(s_np, name="s_const")
            s_stage = small.tile([P, SG * SG], f32, tag="stat_stage")
            nc.sync.dma_start(s_stage[:], s_dram[:])
            S = small.tile([P, SG * SG], f32r, tag="stat")
            nc.vector.tensor_copy(S[:], s_stage[:])

            for xt, ot in ((x_raw, out_raw), (x_rect, out_rect)):
                for g in range(NS // SG):
                    acc = psum.tile([SG, F], f32, tag="acc")
                    for m in range(SG):
                        s = g * SG + m
                        # one contiguous 2 MiB transfer per sample
                        if xt is x_raw and s < 2:
                            t = t_first[s]
                        else:
                            t = pool.tile([P, FF], f32, tag="in")
                            nc.gpsimd.dma_start(t[:], xt[s])
                        # fold channel halves + round to fp32r for the PE
                        tr = redp.tile([P, F], f32r, tag="red")
                        nc.vector.tensor_add(tr[:], t[:, :F], t[:, F:])
                        for j in range(NB):
                            nc.tensor.matmul(
                                acc[:, j * BN : (j + 1) * BN],
                                S[:, SG * m : SG * m + SG],
                                tr[:, j * BN : (j + 1) * BN],
                                start=(m == 0),
                                stop=(m == SG - 1),
                            )
                    osb = small.tile([SG, F], f32, tag="osb")
                    nc.vector.tensor_copy(osb[:], acc[:])
                    nc.sync.dma_start(ot[g * SG : (g + 1) * SG], osb[:])

    nc.compile()
    return nc


def _device_channel_sums(raw, rect, trace=False):
    """Run the bass kernel on 8 cores; return (sums_raw, sums_rect) [128, 2048]
    and the BassKernelResults."""
    from concourse.bass_utils import run_bass_kernel_spmd

    if "nc" not in _CACHE:
        _CACHE["nc"] = _build_nc()
    nc = _CACHE["nc"]

    raw4 = raw.reshape(N_CORES, NS, P, FF)
    rect4 = rect.reshape(N_CORES, NS, P, FF)
    in_maps = [{"x_raw": raw4[i], "x_rect": rect4[i]} for i in range(N_CORES)]
    res = run_bass_kernel_spmd(nc, in_maps, list(range(N_CORES)), trace=trace)

    sums_raw = np.concatenate([res.results[i]["out_raw"] for i in range(N_CORES)])
    sums_rect = np.concatenate([res.results[i]["out_rect"] for i in range(N_CORES)])
    return sums_raw, sums_rect, res


def _rank_from_sums(sums):
    # channel mean (exact: /256 is a power of two), normalize, svd, count
    m = (sums / np.float32(C)).astype(np.float32)
    nrm = np.linalg.norm(m, axis=1, keepdims=True)
    normed = (m / nrm).reshape(-1, H, W)
    s = np.linalg.svd(normed.astype(np.float32), compute_uv=False)
    return (s > 0.0).sum(axis=1).astype(np.float32)


def kernel(raw_feat, rectified_feat, trace=False):
    raw = np.ascontiguousarray(np.asarray(raw_feat, dtype=np.float32))
    rect = np.ascontiguousarray(np.asarray(rectified_feat, dtype=np.float32))

    sums_raw, sums_rect, res = _device_channel_sums(raw, rect, trace=trace)
    _CACHE["last_results"] = res
    _CACHE["last_sums"] = (sums_raw, sums_rect)

    rank1 = _rank_from_sums(sums_raw)
    rank2 = _rank_from_sums(sums_rect)
    loss = np.maximum(np.float32(0.0), -(rank1 - rank2))
    loss = loss.sum(dtype=np.float32) / np.float32(raw.shape[0])
    return np.asarray(loss, dtype=np.float32)


# revision 4
# speedup vs baseline: 1.0023x; 1.0023x over previous
"""Bass/Trainium2 kernel for nn_LowRankLoss.

Reference computation:
  m      = mean(feat, axis=1)                      # [n, h, w], channel mean
  normed = m / ||m||_F (per sample)
  rank   = #(singular values of normed > 0)        # [n]
  loss   = sum(max(0, -(rank1 - rank2))) / n

The memory-bound part (target_regime=memory) is the channel-mean reduction
over two [128, 256, 32, 64] f32 tensors (512 MiB total). That runs on 8
NeuronCores, data-parallel over the batch dim (16 samples/core). The device
returns per-sample channel sums [n, 2048]; the tiny per-sample SVDs
(128 matrices of 32x64) and the scalar loss are finished on host.

Device design per core (per input tensor, viewed [NS=16, 2, 128, F=2048]):
  - Two contiguous 1 MiB DMAs per sample -> SBUF [128, 2048] x2. 8 KiB
    per-partition descriptors: measured optimal (16 KiB descriptors
    trigger a ~20% slowdown on SDMA engine 15 - the SWDGE descriptor-ring
    port pathology - and the statically balanced work then drags the whole
    stream from 164 us to 199 us). SWDGE (gpsimd) issues the steady-state
    input DMAs across all 16 SDMA engines; sample 0 rides the HWDGE rings
    (sync/scalar, shorter first-byte) while the Q7 SWDGE path spins up,
    and is issued BEFORE the one-hot stationary load so input bytes flow
    at the earliest point.
  - VectorE folds the two channel halves (t0 + t1) and rounds to fp32r
    for the PE (fp32r moving streams 1 cycle/row vs 4 for fp32).
  - TensorE reduces the remaining 128 channels (partition dim) per sample:
    stationary S_m [128, 8] is all-ones in column m = s%8 and zero
    elsewhere, so sample s lands in PSUM row m while other rows accumulate
    +0. Eight samples share one PSUM accumulation group per 512-col bank.
  - acc -> SBUF via VectorE -> DMA out per group.
  - Tail: everything after the last input byte is serial latency, so the
    globally last sample (x_rect s15) is pipelined in 512-column chunks:
    chunked strided DMAs -> chunked fold -> chunked stop-matmul into
    per-bank PSUM tiles -> chunked PSUM->SBUF copies -> chunked output
    DMAs (final chunk on the scalar HWDGE queue so it does not FIFO
    behind the earlier output chunks). This cuts the post-stream serial
    chain from ~9.6 us (2.3 ADD + 1.9 MM + 2.3 COPY + 1.2 out-DMA) to
    ~3.5 us. The remaining ~8.5 us after the last output DMA is the
    framework's fixed epilogue (DMA-lane drains, two all-engine barriers,
    per-engine semaphore-file resets) and does not depend on the kernel
    body.
fp32r truncates the data mantissa (~1e-4 rel err), far below what could
flip a singular-value-positivity count (min sigma ~2e-2 here).
"""

import numpy as np

N_CORES = 8
NS = 16           # samples per core
C = 256           # channels
H, W = 32, 64
F = H * W         # 2048 spatial
CB = 2            # channel halves
P = 128           # partitions
SG = 8            # samples per PSUM group
NB = 4            # matmuls per sample (N=512 PSUM bank limit)
BN = F // NB      # 512

_CACHE = {}


def _build_nc():
    import concourse.bacc as bacc
    import concourse.mybir as mybir
    import concourse.tile as tile

    nc = bacc.Bacc(None, target_bir_lowering=False)
    f32 = mybir.dt.float32
    f32r = mybir.dt.float32r

    x_raw = nc.dram_tensor("x_raw", [NS, CB, P, F], f32, kind="ExternalInput")
    x_rect = nc.dram_tensor("x_rect", [NS, CB, P, F], f32, kind="ExternalInput")
    out_raw = nc.dram_tensor("out_raw", [NS, F], f32, kind="ExternalOutput")
    out_rect = nc.dram_tensor("out_rect", [NS, F], f32, kind="ExternalOutput")

    with tile.TileContext(nc) as tc:
        with (
            tc.tile_pool(name="io", bufs=8) as pool,
            tc.tile_pool(name="red", bufs=3) as redp,
            tc.tile_pool(name="small", bufs=2) as small,
            tc.tile_pool(name="psum", bufs=1, space="PSUM") as psum,
        ):
            # Sample 0 rides the HWDGE rings before anything else so input
            # bytes start flowing at the earliest possible point.
            t0_first = pool.tile([P, F], f32, tag="in0")
            t1_first = pool.tile([P, F], f32, tag="in1")
            nc.sync.dma_start(t0_first[:], x_raw[0, 0])
            nc.scalar.dma_start(t1_first[:], x_raw[0, 1])

            # C[k, 8m + j] = 1 if j == m else 0; lhsT for sample s is the
            # [128, 8] slice C[:, 8m:8m+8] with m = s % 8.
            s_np = np.zeros((P, SG * SG), np.float32)
            for m in range(SG):
                s_np[:, SG * m + m] = 1.0
            s_dram = nc.inline_tensor(s_np, name="s_const")
            s_stage = small.tile([P, SG * SG], f32, tag="stat_stage")
            nc.sync.dma_start(s_stage[:], s_dram[:])
            S = small.tile([P, SG * SG], f32r, tag="stat")
            nc.vector.tensor_copy(S[:], s_stage[:])

            for xt, ot in ((x_raw, out_raw), (x_rect, out_rect)):
                for g in range(NS // SG):
                    last_group = xt is x_rect and g == NS // SG - 1
                    if not last_group:
                        acc = psum.tile([SG, F], f32, tag="acc", bufs=1)
                    else:
                        # per-bank PSUM tiles so each 512-col chunk's copy
                        # only waits on its own accumulation chain
                        accc = [
                            psum.tile([SG, BN], f32, tag=f"accc{j}", bufs=1,
                                      name=f"accc{j}")
                            for j in range(NB)
                        ]
                    for m in range(SG):
                        s = g * SG + m
                        if last_group and m == SG - 1:
                            break  # chunked tail below
                        if s == 0 and xt is x_raw:
                            t0, t1 = t0_first, t1_first
                        else:
                            t0 = pool.tile([P, F], f32, tag="in0")
                            t1 = pool.tile([P, F], f32, tag="in1")
                            nc.gpsimd.dma_start(t0[:], xt[s, 0])
                            nc.gpsimd.dma_start(t1[:], xt[s, 1])
                        # fold channel halves + round to fp32r for the PE
                        tr = redp.tile([P, F], f32r, tag="red")
                        nc.vector.tensor_add(tr[:], t0[:], t1[:])
                        for j in range(NB):
                            dst = acc[:, j * BN : (j + 1) * BN] if not last_group else accc[j][:]
                            nc.tensor.matmul(
                                dst,
                                S[:, SG * m : SG * m + SG],
                                tr[:, j * BN : (j + 1) * BN],
                                start=(m == 0),
                                stop=(m == SG - 1) and not last_group,
                            )
                    if not last_group:
                        osb = small.tile([SG, F], f32, tag="osb")
                        nc.vector.tensor_copy(osb[:], acc[:])
                        nc.sync.dma_start(ot[g * SG : (g + 1) * SG], osb[:])
                    else:
                        # ---- chunked tail for the globally last sample ----
                        s = g * SG + SG - 1
                        m = SG - 1
                        rows = ot[g * SG : (g + 1) * SG]
                        for j in range(NB):
                            c0, c1 = j * BN, (j + 1) * BN
                            ta = pool.tile([P, BN], f32, tag="chka", bufs=4)
                            tb = pool.tile([P, BN], f32, tag="chkb", bufs=4)
                            nc.gpsimd.dma_start(ta[:], xt[s, 0, :, c0:c1])
                            nc.gpsimd.dma_start(tb[:], xt[s, 1, :, c0:c1])
                            trc = redp.tile([P, BN], f32r, tag="redc", bufs=4)
                            nc.vector.tensor_add(trc[:], ta[:], tb[:])
                            nc.tensor.matmul(
                                accc[j][:],
                                S[:, SG * m : SG * m + SG],
                                trc[:],
                                start=False,
                                stop=True,
                            )
                            osbc = small.tile([SG, BN], f32, tag="osbc", bufs=4)
                            nc.vector.tensor_copy(osbc[:], accc[j][:])
                            # final chunk takes the other HWDGE queue so it
                            # doesn't serialize behind chunks 0-2
                            eng = nc.scalar if j == NB - 1 else nc.sync
                            eng.dma_start(rows[:, c0:c1], osbc[:])

    nc.compile()
    return nc


def _device_channel_sums(raw, rect, trace=False):
    """Run the bass kernel on 8 cores; return (sums_raw, sums_rect) [128, 2048]
    and the BassKernelResults."""
    from concourse.bass_utils import run_bass_kernel_spmd

    if "nc" not in _CACHE:
        _CACHE["nc"] = _build_nc()
    nc = _CACHE["nc"]

    raw5 = raw.reshape(N_CORES, NS, CB, P, F)
    rect5 = rect.reshape(N_CORES, NS, CB, P, F)
    in_maps = [{"x_raw": raw5[i], "x_rect": rect5[i]} for i in range(N_CORES)]
    res = run_bass_kernel_spmd(nc, in_maps, list(range(N_CORES)), trace=trace)

    sums_raw = np.concatenate([res.results[i]["out_raw"] for i in range(N_CORES)])
    sums_rect = np.concatenate([res.results[i]["out_rect"] for i in range(N_CORES)])
    return sums_raw, sums_rect, res


def _rank_from_sums(sums):
    # channel mean (exact: /256 is a power of two), normalize, svd, count
    m = (sums / np.float32(C)).astype(np.float32)
    nrm = np.linalg.norm(m, axis=1, keepdims=True)
    normed = (m / nrm).reshape(-1, H, W)
    s = np.linalg.svd(normed.astype(np.float32), compute_uv=False)
    return (s > 0.0).sum(axis=1).astype(np.float32)


def kernel(raw_feat, rectified_feat, trace=False):
    raw = np.ascontiguousarray(np.asarray(raw_feat, dtype=np.float32))
    rect = np.ascontiguousarray(np.asarray(rectified_feat, dtype=np.float32))

    sums_raw, sums_rect, res = _device_channel_sums(raw, rect, trace=trace)
    _CACHE["last_results"] = res
    _CACHE["last_sums"] = (sums_raw, sums_rect)

    rank1 = _rank_from_sums(sums_raw)
    rank2 = _rank_from_sums(sums_rect)
    loss = np.maximum(np.float32(0.0), -(rank1 - rank2))
    loss = loss.sum(dtype=np.float32) / np.float32(raw.shape[0])
    return np.asarray(loss, dtype=np.float32)
